# revision 1
# baseline (speedup 1.0000x reference)
import sys
sys.path.insert(0, '/opt/trn_rl_repo')
import numpy as np

from concourse import bass, mybir, bacc
from concourse.tile import TileContext
from concourse.masks import make_identity
from concourse import bass_utils

# ---- problem constants (hardcoded) ----
D = 64
H = 8
L = 5
NP = 4            # points
DH = 8
NQ = 20000
B = 2
LIN = 45109
SS = [(184, 184), (92, 92), (46, 46), (23, 23), (12, 12)]   # (Hl, Wl)
LSI = [0, 33856, 42320, 44436, 44965]
STRIDE = 187                      # padded row stride in cells (>= Wl+3 for all levels)
ROWS = [h + 3 for (h, w) in SS]   # zero-padded rows per level (y0c,y0c+1 <= Hl+2)
LBROW = [0]
for r in ROWS[:-1]:
    LBROW.append(LBROW[-1] + r)
CELLS = sum(ROWS) * STRIDE        # 372*187 = 69564
NCH = 544                         # 128-token chunks: 544*128 = 69632
PADC = NCH * 128                  # padded cells per head
NQP = 5120                        # queries per core (padded)
NT = NQP // 128                   # 40 query tiles
NS = H * L * NP                   # 160 sample slots per query
FP32 = mybir.dt.float32
BF16 = mybir.dt.bfloat16
INT32 = mybir.dt.int32
AX = mybir.AluOpType
AF = mybir.ActivationFunctionType


def _build_tables():
    # per-slot (j = h*20 + l*4 + p) constant rows
    t_wl = np.zeros(NS, np.float32)
    t_hl = np.zeros(NS, np.float32)
    t_cxhi = np.zeros(NS, np.float32)
    t_cyhi = np.zeros(NS, np.float32)
    t_base = np.zeros(NS, np.float32)
    for h in range(H):
        for l in range(L):
            hl, wl = SS[l]
            for p in range(NP):
                j = h * (L * NP) + l * NP + p
                t_wl[j] = wl
                t_hl[j] = hl
                t_cxhi[j] = wl + 1
                t_cyhi[j] = hl + 1
                t_base[j] = h * PADC + LBROW[l] * STRIDE
    return t_wl, t_hl, t_cxhi, t_cyhi, t_base


def build_program(nt=NT):
    nc = bacc.Bacc()
    dt = nc.dram_tensor
    vT = dt("vT", (D + 1, PADC), FP32, kind="ExternalInput")
    qfT = dt("qfT", (D, NQP), FP32, kind="ExternalInput")
    qpT = dt("qpT", (D, NQP), FP32, kind="ExternalInput")
    qf = dt("qf", (NQP, D), FP32, kind="ExternalInput")
    ref = dt("ref", (NQP, 2), FP32, kind="ExternalInput")
    Wv = dt("Wv", (D + 1, D), FP32, kind="ExternalInput")
    Wo = dt("Wo", (D, H * L * NP * 2), FP32, kind="ExternalInput")
    Wa = dt("Wa", (D, NS), FP32, kind="ExternalInput")
    Wout = dt("Wout", (D, D), FP32, kind="ExternalInput")
    W1 = dt("W1", (D, 1024), FP32, kind="ExternalInput")
    W2 = dt("W2", (128, 8 * D), FP32, kind="ExternalInput")
    bo_r = dt("bo_r", (128, 320), FP32, kind="ExternalInput")
    ba_r = dt("ba_r", (128, NS), FP32, kind="ExternalInput")
    bout_r = dt("bout_r", (128, D), FP32, kind="ExternalInput")
    g1_r = dt("g1_r", (128, D), FP32, kind="ExternalInput")
    b1_r = dt("b1_r", (128, D), FP32, kind="ExternalInput")
    g2_r = dt("g2_r", (128, D), FP32, kind="ExternalInput")
    b2_r = dt("b2_r", (128, D), FP32, kind="ExternalInput")
    bff1_c = dt("bff1_c", (128, 8), FP32, kind="ExternalInput")
    bff2_r = dt("bff2_r", (128, D), FP32, kind="ExternalInput")
    t_wl = dt("t_wl", (128, NS), FP32, kind="ExternalInput")
    t_hl = dt("t_hl", (128, NS), FP32, kind="ExternalInput")
    t_cxhi = dt("t_cxhi", (128, NS), FP32, kind="ExternalInput")
    t_cyhi = dt("t_cyhi", (128, NS), FP32, kind="ExternalInput")
    t_base = dt("t_base", (128, NS), FP32, kind="ExternalInput")
    OUT = dt("out", (NQP, D), FP32, kind="ExternalOutput")
    V4 = dt("v4", (H * PADC, DH), BF16, kind="Internal")

    with TileContext(nc) as tc:
        with tc.tile_pool(name="const", bufs=1) as cp:
            def ld(src, shape, dtype=FP32):
                t = cp.tile(shape, dtype, tag=src.name + "_sb")
                nc.sync.dma_start(t[:], src[:])
                return t
            Wv_sb = ld(Wv, [D + 1, D])
            Wo_sb = ld(Wo, [D, 320])
            Wa_sb = ld(Wa, [D, NS])
            Wout_sb = ld(Wout, [D, D])
            W1_sb = ld(W1, [D, 1024])
            W2_sb = ld(W2, [128, 8 * D])
            bo_sb = ld(bo_r, [128, 320])
            ba_sb = ld(ba_r, [128, NS])
            bout_sb = ld(bout_r, [128, D])
            g1_sb = ld(g1_r, [128, D])
            b1_sb = ld(b1_r, [128, D])
            g2_sb = ld(g2_r, [128, D])
            b2_sb = ld(b2_r, [128, D])
            bff1_sb = ld(bff1_c, [128, 8])
            bff2_sb = ld(bff2_r, [128, D])
            twl_sb = ld(t_wl, [128, NS])
            thl_sb = ld(t_hl, [128, NS])
            tcx_sb = ld(t_cxhi, [128, NS])
            tcy_sb = ld(t_cyhi, [128, NS])
            tbase_sb = ld(t_base, [128, NS])
            eps_sb = cp.tile([128, 1], FP32, tag="eps")
            nc.vector.memset(eps_sb[:], 1e-5)
            ident = cp.tile([128, 128], FP32, tag="ident")
            make_identity(nc, ident[:])
            # qT = qfT + qpT
            qT = cp.tile([D, NQP], FP32, tag="qT")
            with tc.tile_pool(name="vstage_pool", bufs=1) as vsp, \
                 tc.tile_pool(name="vload", bufs=3) as vl, \
                 tc.tile_pool(name="vps", bufs=4, space="PSUM") as vps:
                qfT_sb = vsp.tile([D, NQP], FP32, tag="qfT_sb")
                nc.sync.dma_start(qfT_sb[:], qfT[:])
                nc.sync.dma_start(qT[:], qpT[:])
                nc.vector.tensor_tensor(out=qT[:], in0=qT[:], in1=qfT_sb[:], op=AX.add)

                # ---------- value pipeline ----------
                stage = vsp.tile([128, NCH * D], BF16, tag="vstage")
                CK = 16  # 128-token chunks per load (2048 tokens)
                for g in range(NCH // CK):      # 34 groups
                    vchunk = vl.tile([D + 1, CK * 128], FP32, tag="vchunk")
                    nc.sync.dma_start(vchunk[:], vT[:, g * CK * 128:(g + 1) * CK * 128])
                    for q4 in range(CK // 4):   # 4 psum groups per load
                        ps = vps.tile([128, 256], FP32, tag="vps")
                        for j in range(4):
                            kc = q4 * 4 + j
                            nc.tensor.matmul(
                                out=ps[:, j * 64:(j + 1) * 64],
                                lhsT=vchunk[:, kc * 128:(kc + 1) * 128],
                                rhs=Wv_sb[:],
                                start=True, stop=True,
                            )
                        kc0 = g * CK + q4 * 4
                        nc.scalar.activation(
                            out=stage[:, kc0 * 64:(kc0 + 4) * 64],
                            in_=ps[:], func=AF.Copy,
                        )
                # 8 per-head DMAs into V4 (dest [cell, dh] contiguous per head)
                st_v = stage[:].rearrange("p (kc c) -> p kc c", c=64)
                v4_v = V4[:].rearrange("(h kc p) d -> h p kc d", h=H, kc=NCH, p=128)
                for h in range(H):
                    nc.sync.dma_start(v4_v[h], st_v[:, :, h * 8:(h + 1) * 8])

            # ---------- query loop ----------
            with tc.tile_pool(name="qw", bufs=2) as qp, \
                 tc.tile_pool(name="qg", bufs=3) as qg, \
                 tc.tile_pool(name="qps", bufs=1, space="PSUM") as qps:
                for t in range(nt):
                    qs = slice(t * 128, (t + 1) * 128)
                    # attention weights (softmax over 20 per head)
                    ps_aw = qps.tile([128, NS], FP32, tag="ps_aw")
                    nc.tensor.matmul(out=ps_aw[:], lhsT=qT[:, qs], rhs=Wa_sb[:], start=True, stop=True)
                    logit = qp.tile([128, NS], FP32, tag="logit")
                    nc.vector.tensor_tensor(out=logit[:], in0=ps_aw[:], in1=ba_sb[:], op=AX.add)
                    mx = qp.tile([128, H], FP32, tag="mx")
                    lv = logit[:].rearrange("p (h k) -> p h k", h=H)
                    nc.vector.tensor_reduce(out=mx[:], in_=lv, axis=mybir.AxisListType.X, op=AX.max)
                    mxb = mx[:].rearrange("p (h one) -> p h one", one=1).to_broadcast((128, H, L * NP))
                    ls = qp.tile([128, NS], FP32, tag="ls")
                    nc.vector.tensor_tensor(out=ls[:].rearrange("p (h k) -> p h k", h=H), in0=lv, in1=mxb, op=AX.subtract)
                    ee = qp.tile([128, NS], FP32, tag="ee")
                    nc.scalar.activation(out=ee[:], in_=ls[:], func=AF.Exp)
                    sm = qp.tile([128, H], FP32, tag="sm")
                    nc.vector.tensor_reduce(out=sm[:], in_=ee[:].rearrange("p (h k) -> p h k", h=H), axis=mybir.AxisListType.X, op=AX.add)
                    rc = qp.tile([128, H], FP32, tag="rc")
                    nc.vector.reciprocal(out=rc[:], in_=sm[:])
                    aw = qp.tile([128, NS], FP32, tag="aw")
                    rcb = rc[:].rearrange("p (h one) -> p h one", one=1).to_broadcast((128, H, L * NP))
                    nc.vector.tensor_tensor(out=aw[:].rearrange("p (h k) -> p h k", h=H), in0=ee[:].rearrange("p (h k) -> p h k", h=H), in1=rcb, op=AX.mult)

                    # sampling offsets
                    ps_off = qps.tile([128, 320], FP32, tag="ps_off")
                    nc.tensor.matmul(out=ps_off[:], lhsT=qT[:, qs], rhs=Wo_sb[:], start=True, stop=True)
                    off = qp.tile([128, 320], FP32, tag="off")
                    nc.vector.tensor_tensor(out=off[:], in0=ps_off[:], in1=bo_sb[:], op=AX.add)

                    reft = qp.tile([128, 2], FP32, tag="reft")
                    nc.sync.dma_start(reft[:], ref[qs, :])
                    refx = reft[:, 0:1].to_broadcast((128, NS))
                    refy = reft[:, 1:2].to_broadcast((128, NS))

                    # positions: p = ref*W + off + 0.5, clamp [0, W+1]
                    tmp = qp.tile([128, NS], FP32, tag="tmp")
                    pxc = qp.tile([128, NS], FP32, tag="pxc")
                    pyc = qp.tile([128, NS], FP32, tag="pyc")
                    nc.vector.tensor_tensor(out=tmp[:], in0=refx, in1=twl_sb[:], op=AX.mult)
                    nc.vector.scalar_tensor_tensor(out=tmp[:], in0=off[:, 0::2], scalar=0.5, in1=tmp[:], op0=AX.add, op1=AX.add)
                    nc.vector.scalar_tensor_tensor(out=pxc[:], in0=tmp[:], scalar=0.0, in1=tcx_sb[:], op0=AX.max, op1=AX.min)
                    nc.vector.tensor_tensor(out=tmp[:], in0=refy, in1=thl_sb[:], op=AX.mult)
                    nc.vector.scalar_tensor_tensor(out=tmp[:], in0=off[:, 1::2], scalar=0.5, in1=tmp[:], op0=AX.add, op1=AX.add)
                    nc.vector.scalar_tensor_tensor(out=pyc[:], in0=tmp[:], scalar=0.0, in1=tcy_sb[:], op0=AX.max, op1=AX.min)

                    x0i = qp.tile([128, NS], INT32, tag="x0i")
                    x0f = qp.tile([128, NS], FP32, tag="x0f")
                    y0i = qp.tile([128, NS], INT32, tag="y0i")
                    y0f = qp.tile([128, NS], FP32, tag="y0f")
                    nc.scalar.activation(out=x0i[:], in_=pxc[:], func=AF.Copy)
                    nc.scalar.activation(out=x0f[:], in_=x0i[:], func=AF.Copy)
                    nc.scalar.activation(out=y0i[:], in_=pyc[:], func=AF.Copy)
                    nc.scalar.activation(out=y0f[:], in_=y0i[:], func=AF.Copy)
                    fx = qp.tile([128, NS], FP32, tag="fx")
                    fy = qp.tile([128, NS], FP32, tag="fy")
                    nc.vector.tensor_tensor(out=fx[:], in0=pxc[:], in1=x0f[:], op=AX.subtract)
                    nc.vector.tensor_tensor(out=fy[:], in0=pyc[:], in1=y0f[:], op=AX.subtract)

                    # gather cell index: base + y0*187 + x0 (exact in fp32)
                    gfv = qp.tile([128, NS], FP32, tag="gfv")
                    nc.vector.scalar_tensor_tensor(out=gfv[:], in0=y0f[:], scalar=float(STRIDE), in1=tbase_sb[:], op0=AX.mult, op1=AX.add)
                    nc.vector.tensor_tensor(out=gfv[:], in0=gfv[:], in1=x0f[:], op=AX.add)
                    idxs = qp.tile([128, 2 * NS], INT32, tag="idxs")
                    nc.scalar.activation(out=idxs[:, 0::2], in_=gfv[:], func=AF.Copy)
                    nc.vector.tensor_scalar(out=idxs[:, 1::2], in0=gfv[:], scalar1=float(STRIDE), scalar2=None, op0=AX.add)

                    # fused bilinear*attention weights
                    tt = qp.tile([128, NS], FP32, tag="tt")
                    a0 = qp.tile([128, NS], FP32, tag="a0")
                    u0 = qp.tile([128, NS], FP32, tag="u0")
                    u1 = qp.tile([128, NS], FP32, tag="u1")
                    w00 = qp.tile([128, NS], BF16, tag="w00")
                    w01 = qp.tile([128, NS], BF16, tag="w01")
                    w10 = qp.tile([128, NS], BF16, tag="w10")
                    w11 = qp.tile([128, NS], BF16, tag="w11")
                    nc.vector.tensor_tensor(out=tt[:], in0=aw[:], in1=fy[:], op=AX.mult)
                    nc.vector.tensor_tensor(out=a0[:], in0=aw[:], in1=tt[:], op=AX.subtract)
                    nc.vector.tensor_tensor(out=u0[:], in0=a0[:], in1=fx[:], op=AX.mult)
                    nc.vector.tensor_tensor(out=u1[:], in0=tt[:], in1=fx[:], op=AX.mult)
                    nc.vector.tensor_tensor(out=w00[:], in0=a0[:], in1=u0[:], op=AX.subtract)
                    nc.vector.tensor_tensor(out=w10[:], in0=tt[:], in1=u1[:], op=AX.subtract)
                    nc.scalar.activation(out=w01[:], in_=u0[:], func=AF.Copy)
                    nc.scalar.activation(out=w11[:], in_=u1[:], func=AF.Copy)

                    # gather: per index 16 elems = 2 cells x 8 dh.
                    # one indirect DMA is limited to ~8191 descriptors (16-bit
                    # ring sem), so chunk the 320 per-row indices.
                    G = qg.tile([128, 2 * NS * 16], BF16, tag="G")
                    CHUNK = 64  # 128*64 = 8192 desc > ring limit; 63 used below
                    CHUNK = 63  # 128*63 = 8064 descriptors per call (limit ~8191)
                    for c0 in range(0, 2 * NS, CHUNK):
                        c1 = min(c0 + CHUNK, 2 * NS)
                        nc.gpsimd.indirect_dma_start(
                            out=G[:, c0 * 16:c1 * 16], out_offset=None,
                            in_=V4[:],
                            in_offset=bass.IndirectOffsetOnAxis(ap=idxs[:, c0:c1], axis=0),
                        )

                    # blend: m[p, h, d, lp] = sum_{r,c} w_rc * G[p,(h,lp),r,c,d]
                    gv = G[:].rearrange("p (h lp r c d) -> p h lp r c d", h=H, lp=L * NP, r=2, c=2, d=DH)
                    m = qp.tile([128, H * DH * L * NP], FP32, tag="m")
                    m2 = qp.tile([128, H * DH * L * NP], FP32, tag="m2")
                    mv = m[:].rearrange("p (h d lp) -> p h lp d", h=H, d=DH, lp=L * NP)
                    m2v = m2[:].rearrange("p (h d lp) -> p h lp d", h=H, d=DH, lp=L * NP)
                    def wb(w):
                        return w[:].rearrange("p (h lp one) -> p h lp one", h=H, one=1).to_broadcast((128, H, L * NP, DH))
                    nc.vector.tensor_tensor(out=mv, in0=gv[:, :, :, 0, 0, :], in1=wb(w00), op=AX.mult)
                    nc.vector.tensor_tensor(out=m2v, in0=gv[:, :, :, 0, 1, :], in1=wb(w01), op=AX.mult)
                    nc.vector.tensor_tensor(out=m[:], in0=m[:], in1=m2[:], op=AX.add)
                    nc.vector.tensor_tensor(out=m2v, in0=gv[:, :, :, 1, 0, :], in1=wb(w10), op=AX.mult)
                    nc.vector.tensor_tensor(out=m[:], in0=m[:], in1=m2[:], op=AX.add)
                    nc.vector.tensor_tensor(out=m2v, in0=gv[:, :, :, 1, 1, :], in1=wb(w11), op=AX.mult)
                    nc.vector.tensor_tensor(out=m[:], in0=m[:], in1=m2[:], op=AX.add)
                    attn = qp.tile([128, D], FP32, tag="attn")
                    nc.vector.tensor_reduce(
                        out=attn[:], in_=m[:].rearrange("p (hd lp) -> p hd lp", lp=L * NP),
                        axis=mybir.AxisListType.X, op=AX.add,
                    )

                    # output projection + residual + LN1
                    ps_t = qps.tile([64, 128], FP32, tag="ps_t")
                    nc.tensor.transpose(out=ps_t[:], in_=attn[:], identity=ident[:])
                    attnT = qp.tile([64, 128], FP32, tag="attnT")
                    nc.scalar.activation(out=attnT[:], in_=ps_t[:], func=AF.Copy)
                    ps_ao = qps.tile([128, D], FP32, tag="ps_ao")
                    nc.tensor.matmul(out=ps_ao[:], lhsT=attnT[:], rhs=Wout_sb[:], start=True, stop=True)
                    qft = qp.tile([128, D], FP32, tag="qft")
                    nc.sync.dma_start(qft[:], qf[qs, :])
                    xpre = qp.tile([128, D], FP32, tag="xpre")
                    nc.vector.tensor_tensor(out=xpre[:], in0=ps_ao[:], in1=bout_sb[:], op=AX.add)
                    nc.vector.tensor_tensor(out=xpre[:], in0=xpre[:], in1=qft[:], op=AX.add)

                    def layernorm(xin, gg, bb, xout_tag):
                        s1 = qp.tile([128, 1], FP32, tag=xout_tag + "_s1")
                        nc.vector.tensor_reduce(out=s1[:], in_=xin[:], axis=mybir.AxisListType.X, op=AX.add)
                        mn = qp.tile([128, 1], FP32, tag=xout_tag + "_mn")
                        nc.vector.tensor_scalar_mul(out=mn[:], in0=s1[:], scalar1=1.0 / 64.0)
                        xc = qp.tile([128, D], FP32, tag=xout_tag + "_xc")
                        nc.vector.tensor_tensor(out=xc[:], in0=xin[:], in1=mn[:].to_broadcast((128, D)), op=AX.subtract)
                        sq = qp.tile([128, D], FP32, tag=xout_tag + "_sq")
                        nc.scalar.activation(out=sq[:], in_=xc[:], func=AF.Square)
                        s2 = qp.tile([128, 1], FP32, tag=xout_tag + "_s2")
                        nc.vector.tensor_reduce(out=s2[:], in_=sq[:], axis=mybir.AxisListType.X, op=AX.add)
                        s2m = qp.tile([128, 1], FP32, tag=xout_tag + "_s2m")
                        nc.vector.tensor_scalar_mul(out=s2m[:], in0=s2[:], scalar1=1.0 / 64.0)
                        std = qp.tile([128, 1], FP32, tag=xout_tag + "_std")
                        nc.scalar.activation(out=std[:], in_=s2m[:], func=AF.Sqrt, bias=eps_sb[:])
                        rstd = qp.tile([128, 1], FP32, tag=xout_tag + "_rstd")
                        nc.vector.reciprocal(out=rstd[:], in_=std[:])
                        xo = qp.tile([128, D], FP32, tag=xout_tag)
                        nc.vector.tensor_tensor(out=xo[:], in0=xc[:], in1=rstd[:].to_broadcast((128, D)), op=AX.mult)
                        nc.vector.tensor_tensor(out=xo[:], in0=xo[:], in1=gg[:], op=AX.mult)
                        nc.vector.tensor_tensor(out=xo[:], in0=xo[:], in1=bb[:], op=AX.add)
                        return xo

                    x1 = layernorm(xpre, g1_sb, b1_sb, "x1")

                    # FFN
                    ps_t2 = qps.tile([64, 128], FP32, tag="ps_t2")
                    nc.tensor.transpose(out=ps_t2[:], in_=x1[:], identity=ident[:])
                    x1T = qp.tile([64, 128], FP32, tag="x1T")
                    nc.scalar.activation(out=x1T[:], in_=ps_t2[:], func=AF.Copy)
                    h1 = qp.tile([128, 1024], FP32, tag="h1")
                    for k in range(8):
                        ps_h1 = qps.tile([128, 128], FP32, tag="ps_h1")
                        nc.tensor.matmul(out=ps_h1[:], lhsT=W1_sb[:, k * 128:(k + 1) * 128], rhs=x1T[:], start=True, stop=True)
                        nc.scalar.activation(out=h1[:, k * 128:(k + 1) * 128], in_=ps_h1[:], func=AF.Relu, bias=bff1_sb[:, k:k + 1])
                    ps_h2 = qps.tile([128, D], FP32, tag="ps_h2")
                    for k in range(8):
                        nc.tensor.matmul(out=ps_h2[:], lhsT=h1[:, k * 128:(k + 1) * 128], rhs=W2_sb[:, k * D:(k + 1) * D], start=(k == 0), stop=(k == 7))
                    x2p = qp.tile([128, D], FP32, tag="x2p")
                    nc.vector.tensor_tensor(out=x2p[:], in0=ps_h2[:], in1=bff2_sb[:], op=AX.add)
                    nc.vector.tensor_tensor(out=x2p[:], in0=x2p[:], in1=x1[:], op=AX.add)
                    x2 = layernorm(x2p, g2_sb, b2_sb, "x2")
                    nc.sync.dma_start(OUT[qs, :], x2[:])

    nc.finalize()
    return nc


def _prep_core_inputs(q_feat_b, q_pos_b, ref_b, voxel_b, w):
    # q_* : [5000, 64] shards of one batch; voxel_b: [LIN, 64] of that batch
    qf = np.zeros((NQP, D), np.float32)
    qf[:q_feat_b.shape[0]] = q_feat_b
    qp = np.zeros((NQP, D), np.float32)
    qp[:q_pos_b.shape[0]] = q_pos_b
    rf = np.zeros((NQP, 2), np.float32)
    rf[:ref_b.shape[0]] = ref_b
    # padded-cell voxel layout with mask feature
    va = np.zeros((PADC, D + 1), np.float32)
    for l in range(L):
        hl, wl = SS[l]
        r0 = LBROW[l]
        blk = voxel_b[LSI[l]:LSI[l] + hl * wl].reshape(hl, wl, D)
        rows = (np.arange(hl) + r0 + 1)[:, None] * STRIDE + (np.arange(wl) + 1)[None, :]
        va[rows.ravel(), :D] = blk.reshape(-1, D)
        va[rows.ravel(), D] = 1.0
    m = {
        "vT": np.ascontiguousarray(va.T),
        "qfT": np.ascontiguousarray(qf.T),
        "qpT": np.ascontiguousarray(qp.T),
        "qf": qf,
        "ref": rf,
    }
    m.update(w)
    return m


def _weights_map(inputs):
    t_wl, t_hl, t_cxhi, t_cyhi, t_base = _build_tables()
    rep = lambda v: np.ascontiguousarray(np.broadcast_to(np.asarray(v, np.float32)[None, :], (128, len(v))))
    w = {
        "Wv": np.concatenate([np.asarray(inputs["Wv"], np.float32), np.asarray(inputs["bv"], np.float32)[None, :]], 0),
        "Wo": np.asarray(inputs["Wo"], np.float32),
        "Wa": np.asarray(inputs["Wa"], np.float32),
        "Wout": np.asarray(inputs["Wout"], np.float32),
        "W1": np.asarray(inputs["W1"], np.float32),
        "W2": np.ascontiguousarray(np.asarray(inputs["W2"], np.float32).reshape(8, 128, 64).transpose(1, 0, 2).reshape(128, 512)),
        "bo_r": rep(np.asarray(inputs["bo"], np.float32)),
        "ba_r": rep(np.asarray(inputs["ba"], np.float32)),
        "bout_r": rep(np.asarray(inputs["bout"], np.float32)),
        "g1_r": rep(np.asarray(inputs["g1"], np.float32)),
        "b1_r": rep(np.asarray(inputs["b1"], np.float32)),
        "g2_r": rep(np.asarray(inputs["g2"], np.float32)),
        "b2_r": rep(np.asarray(inputs["b2"], np.float32)),
        "bff1_c": np.ascontiguousarray(np.asarray(inputs["bff1"], np.float32).reshape(8, 128).T),
        "bff2_r": rep(np.asarray(inputs["bff2"], np.float32)),
        "t_wl": rep(t_wl), "t_hl": rep(t_hl), "t_cxhi": rep(t_cxhi),
        "t_cyhi": rep(t_cyhi), "t_base": rep(t_base),
    }
    return w


_NC_CACHE = {}


def kernel(**inputs) -> np.ndarray:
    if "nc" not in _NC_CACHE:
        _NC_CACHE["nc"] = build_program()
    nc = _NC_CACHE["nc"]
    w = _weights_map(inputs)
    q_feat = np.asarray(inputs["q_feat"], np.float32)
    q_pos = np.asarray(inputs["q_pos"], np.float32)
    ref = np.asarray(inputs["reference_points"], np.float32)
    vox = np.asarray(inputs["dense_voxel_flatten"], np.float32)
    QS = NQ // 4
    in_maps = []
    for c in range(8):
        b = c // 4
        s = slice((c % 4) * QS, (c % 4 + 1) * QS)
        in_maps.append(_prep_core_inputs(q_feat[b, s], q_pos[b, s], ref[b, s], vox[b], w))
    res = bass_utils.run_bass_kernel_spmd(nc, in_maps, core_ids=list(range(8)))
    out = np.zeros((B, NQ, D), np.float32)
    for c in range(8):
        b = c // 4
        s = slice((c % 4) * QS, (c % 4 + 1) * QS)
        out[b, s] = res.results[c]["out"][:QS]
    return out



# revision 11
# speedup vs baseline: 56.5091x; 56.5091x over previous
import sys
sys.path.insert(0, '/opt/trn_rl_repo')
import numpy as np

from concourse import bass, mybir, bacc
from concourse.tile import TileContext
from concourse.masks import make_identity
from concourse import bass_utils

# ---- problem constants (hardcoded) ----
D = 64
H = 8
L = 5
NP = 4            # points
DH = 8
NQ = 20000
B = 2
LIN = 45109
SS = [(184, 184), (92, 92), (46, 46), (23, 23), (12, 12)]   # (Hl, Wl)
LSI = [0, 33856, 42320, 44436, 44965]
STRIDE = 187                      # padded row stride in cells (>= Wl+3 for all levels)
ROWS = [h + 3 for (h, w) in SS]   # zero-padded rows per level (y0c,y0c+1 <= Hl+2)
LBROW = [0]
for r in ROWS[:-1]:
    LBROW.append(LBROW[-1] + r)
CELLS = sum(ROWS) * STRIDE        # 372*187 = 69564
NCH = 544                         # 128-token chunks: 544*128 = 69632
PADC = NCH * 128                  # padded cells per head
NQP = 5120                        # queries per core (padded)
NT = NQP // 128                   # 40 query tiles
NS = H * L * NP                   # 160 sample slots per query
FP32 = mybir.dt.float32
BF16 = mybir.dt.bfloat16
INT32 = mybir.dt.int32
AX = mybir.AluOpType
AF = mybir.ActivationFunctionType


def _build_tables():
    # per-slot (j = h*20 + l*4 + p) constant rows
    t_wl = np.zeros(NS, np.float32)
    t_hl = np.zeros(NS, np.float32)
    t_cxhi = np.zeros(NS, np.float32)
    t_cyhi = np.zeros(NS, np.float32)
    t_base = np.zeros(NS, np.float32)
    for h in range(H):
        for l in range(L):
            hl, wl = SS[l]
            for p in range(NP):
                j = h * (L * NP) + l * NP + p
                t_wl[j] = wl
                t_hl[j] = hl
                t_cxhi[j] = wl + 1
                t_cyhi[j] = hl + 1
                t_base[j] = h * PADC + LBROW[l] * STRIDE
    return t_wl, t_hl, t_cxhi, t_cyhi, t_base


def build_program(nt=NT, no_gather=False, no_vpipe=False, no_scatter=False,
                  gather_chunk=63, no_qcompute=False, const_idx=False,
                  ext_table=False):
    nc = bacc.Bacc()
    dt = nc.dram_tensor
    vT = dt("vT", (D + 1, PADC), FP32, kind="ExternalInput")
    qfT = dt("qfT", (D, NQP), FP32, kind="ExternalInput")
    qpT = dt("qpT", (D, NQP), FP32, kind="ExternalInput")
    qf = dt("qf", (NQP, D), FP32, kind="ExternalInput")
    ref = dt("ref", (NQP, 2), FP32, kind="ExternalInput")
    Wv = dt("Wv", (D + 1, D), FP32, kind="ExternalInput")
    Wo = dt("Wo", (D, H * L * NP * 2), FP32, kind="ExternalInput")
    Wa = dt("Wa", (D, NS), FP32, kind="ExternalInput")
    Wout = dt("Wout", (D, D), FP32, kind="ExternalInput")
    W1 = dt("W1", (D, 1024), FP32, kind="ExternalInput")
    W2 = dt("W2", (128, 8 * D), FP32, kind="ExternalInput")
    bo_r = dt("bo_r", (128, 320), FP32, kind="ExternalInput")
    ba_r = dt("ba_r", (128, NS), FP32, kind="ExternalInput")
    bout_r = dt("bout_r", (128, D), FP32, kind="ExternalInput")
    g1_r = dt("g1_r", (128, D), FP32, kind="ExternalInput")
    b1_r = dt("b1_r", (128, D), FP32, kind="ExternalInput")
    g2_r = dt("g2_r", (128, D), FP32, kind="ExternalInput")
    b2_r = dt("b2_r", (128, D), FP32, kind="ExternalInput")
    bff1_c = dt("bff1_c", (128, 8), FP32, kind="ExternalInput")
    bff2_r = dt("bff2_r", (128, D), FP32, kind="ExternalInput")
    t_wl = dt("t_wl", (128, NS), FP32, kind="ExternalInput")
    t_hl = dt("t_hl", (128, NS), FP32, kind="ExternalInput")
    t_cxhi = dt("t_cxhi", (128, NS), FP32, kind="ExternalInput")
    t_cyhi = dt("t_cyhi", (128, NS), FP32, kind="ExternalInput")
    t_base = dt("t_base", (128, NS), FP32, kind="ExternalInput")
    OUT = dt("out", (NQP, D), FP32, kind="ExternalOutput")
    V4 = dt("v4", (H * PADC, DH), BF16, kind="Internal")
    CIDX = dt("cidx", (128, 2 * NS), INT32, kind="ExternalInput") if const_idx else None
    XTBL = dt("xtbl", (H * PADC, DH), BF16, kind="ExternalInput") if ext_table else None

    with TileContext(nc) as tc:
        with tc.tile_pool(name="const", bufs=1) as cp:
            def ld(src, shape, dtype=FP32):
                t = cp.tile(shape, dtype, tag=src.name + "_sb")
                nc.sync.dma_start(t[:], src[:])
                return t
            Wv_sb = ld(Wv, [D + 1, D])
            Wo_sb = ld(Wo, [D, 320])
            Wa_sb = ld(Wa, [D, NS])
            Wout_sb = ld(Wout, [D, D])
            W1_sb = ld(W1, [D, 1024])
            W2_sb = ld(W2, [128, 8 * D])
            bo_sb = ld(bo_r, [128, 320])
            ba_sb = ld(ba_r, [128, NS])
            bout_sb = ld(bout_r, [128, D])
            g1_sb = ld(g1_r, [128, D])
            b1_sb = ld(b1_r, [128, D])
            g2_sb = ld(g2_r, [128, D])
            b2_sb = ld(b2_r, [128, D])
            bff1_sb = ld(bff1_c, [128, 8])
            bff2_sb = ld(bff2_r, [128, D])
            twl_sb = ld(t_wl, [128, NS])
            thl_sb = ld(t_hl, [128, NS])
            tcx_sb = ld(t_cxhi, [128, NS])
            tcy_sb = ld(t_cyhi, [128, NS])
            tbase_sb = ld(t_base, [128, NS])
            eps_sb = cp.tile([128, 1], FP32, tag="eps")
            nc.vector.memset(eps_sb[:], 1e-5)
            cidx_sb = None
            if const_idx:
                cidx_sb = ld(CIDX, [128, 2 * NS], INT32)
            ident = cp.tile([128, 128], FP32, tag="ident")
            make_identity(nc, ident[:])
            # qT = qfT + qpT
            qT = cp.tile([D, NQP], FP32, tag="qT")
            with tc.tile_pool(name="vstage_pool", bufs=1) as vsp, \
                 tc.tile_pool(name="vload", bufs=3) as vl, \
                 tc.tile_pool(name="vps", bufs=4, space="PSUM") as vps:
                qfT_sb = vsp.tile([D, NQP], FP32, tag="qfT_sb")
                nc.sync.dma_start(qfT_sb[:], qfT[:])
                nc.sync.dma_start(qT[:], qpT[:])
                nc.vector.tensor_tensor(out=qT[:], in0=qT[:], in1=qfT_sb[:], op=AX.add)

                # ---------- value pipeline ----------
                stage = vsp.tile([128, NCH * D], BF16, tag="vstage")
                CK = 16  # 128-token chunks per load (2048 tokens)
                for g in range(0 if no_vpipe else NCH // CK):      # 34 groups
                    vchunk = vl.tile([D + 1, CK * 128], FP32, tag="vchunk")
                    nc.sync.dma_start(vchunk[:], vT[:, g * CK * 128:(g + 1) * CK * 128])
                    for q4 in range(CK // 4):   # 4 psum groups per load
                        ps = vps.tile([128, 256], FP32, tag="vps")
                        for j in range(4):
                            kc = q4 * 4 + j
                            nc.tensor.matmul(
                                out=ps[:, j * 64:(j + 1) * 64],
                                lhsT=vchunk[:, kc * 128:(kc + 1) * 128],
                                rhs=Wv_sb[:],
                                start=True, stop=True,
                            )
                        kc0 = g * CK + q4 * 4
                        nc.scalar.activation(
                            out=stage[:, kc0 * 64:(kc0 + 4) * 64],
                            in_=ps[:], func=AF.Copy,
                        )
                # 8 per-head DMAs into V4 (dest [cell, dh] contiguous per head)
                st_v = stage[:].rearrange("p (kc c) -> p kc c", c=64)
                v4_v = V4[:].rearrange("(h kc p) d -> h p kc d", h=H, kc=NCH, p=128)
                for h in range(0 if (no_vpipe or no_scatter) else H):
                    nc.sync.dma_start(v4_v[h], st_v[:, :, h * 8:(h + 1) * 8])

            # ---------- query loop ----------
            with tc.tile_pool(name="qw", bufs=2) as qp, \
                 tc.tile_pool(name="qg", bufs=3) as qg, \
                 tc.tile_pool(name="qps", bufs=1, space="PSUM") as qps:
                for t in range(nt):
                    qs = slice(t * 128, (t + 1) * 128)
                    # attention weights (softmax over 20 per head)
                    ps_aw = qps.tile([128, NS], FP32, tag="ps_aw")
                    nc.tensor.matmul(out=ps_aw[:], lhsT=qT[:, qs], rhs=Wa_sb[:], start=True, stop=True)
                    logit = qp.tile([128, NS], FP32, tag="logit")
                    nc.vector.tensor_tensor(out=logit[:], in0=ps_aw[:], in1=ba_sb[:], op=AX.add)
                    mx = qp.tile([128, H], FP32, tag="mx")
                    lv = logit[:].rearrange("p (h k) -> p h k", h=H)
                    nc.vector.tensor_reduce(out=mx[:], in_=lv, axis=mybir.AxisListType.X, op=AX.max)
                    mxb = mx[:].rearrange("p (h one) -> p h one", one=1).to_broadcast((128, H, L * NP))
                    ls = qp.tile([128, NS], FP32, tag="ls")
                    nc.vector.tensor_tensor(out=ls[:].rearrange("p (h k) -> p h k", h=H), in0=lv, in1=mxb, op=AX.subtract)
                    ee = qp.tile([128, NS], FP32, tag="ee")
                    nc.scalar.activation(out=ee[:], in_=ls[:], func=AF.Exp)
                    sm = qp.tile([128, H], FP32, tag="sm")
                    nc.vector.tensor_reduce(out=sm[:], in_=ee[:].rearrange("p (h k) -> p h k", h=H), axis=mybir.AxisListType.X, op=AX.add)
                    rc = qp.tile([128, H], FP32, tag="rc")
                    nc.vector.reciprocal(out=rc[:], in_=sm[:])
                    aw = qp.tile([128, NS], FP32, tag="aw")
                    rcb = rc[:].rearrange("p (h one) -> p h one", one=1).to_broadcast((128, H, L * NP))
                    nc.vector.tensor_tensor(out=aw[:].rearrange("p (h k) -> p h k", h=H), in0=ee[:].rearrange("p (h k) -> p h k", h=H), in1=rcb, op=AX.mult)

                    # sampling offsets
                    ps_off = qps.tile([128, 320], FP32, tag="ps_off")
                    nc.tensor.matmul(out=ps_off[:], lhsT=qT[:, qs], rhs=Wo_sb[:], start=True, stop=True)
                    off = qp.tile([128, 320], FP32, tag="off")
                    nc.vector.tensor_tensor(out=off[:], in0=ps_off[:], in1=bo_sb[:], op=AX.add)

                    reft = qp.tile([128, 2], FP32, tag="reft")
                    nc.sync.dma_start(reft[:], ref[qs, :])
                    refx = reft[:, 0:1].to_broadcast((128, NS))
                    refy = reft[:, 1:2].to_broadcast((128, NS))

                    # positions: p = ref*W + off + 0.5, clamp [0, W+1]
                    tmp = qp.tile([128, NS], FP32, tag="tmp")
                    pxc = qp.tile([128, NS], FP32, tag="pxc")
                    pyc = qp.tile([128, NS], FP32, tag="pyc")
                    nc.vector.tensor_tensor(out=tmp[:], in0=refx, in1=twl_sb[:], op=AX.mult)
                    nc.vector.scalar_tensor_tensor(out=tmp[:], in0=off[:, 0::2], scalar=0.5, in1=tmp[:], op0=AX.add, op1=AX.add)
                    nc.vector.scalar_tensor_tensor(out=pxc[:], in0=tmp[:], scalar=0.0, in1=tcx_sb[:], op0=AX.max, op1=AX.min)
                    nc.vector.tensor_tensor(out=tmp[:], in0=refy, in1=thl_sb[:], op=AX.mult)
                    nc.vector.scalar_tensor_tensor(out=tmp[:], in0=off[:, 1::2], scalar=0.5, in1=tmp[:], op0=AX.add, op1=AX.add)
                    nc.vector.scalar_tensor_tensor(out=pyc[:], in0=tmp[:], scalar=0.0, in1=tcy_sb[:], op0=AX.max, op1=AX.min)

                    x0i = qp.tile([128, NS], INT32, tag="x0i")
                    x0f = qp.tile([128, NS], FP32, tag="x0f")
                    y0i = qp.tile([128, NS], INT32, tag="y0i")
                    y0f = qp.tile([128, NS], FP32, tag="y0f")
                    nc.scalar.activation(out=x0i[:], in_=pxc[:], func=AF.Copy)
                    nc.scalar.activation(out=x0f[:], in_=x0i[:], func=AF.Copy)
                    nc.scalar.activation(out=y0i[:], in_=pyc[:], func=AF.Copy)
                    nc.scalar.activation(out=y0f[:], in_=y0i[:], func=AF.Copy)
                    fx = qp.tile([128, NS], FP32, tag="fx")
                    fy = qp.tile([128, NS], FP32, tag="fy")
                    nc.vector.tensor_tensor(out=fx[:], in0=pxc[:], in1=x0f[:], op=AX.subtract)
                    nc.vector.tensor_tensor(out=fy[:], in0=pyc[:], in1=y0f[:], op=AX.subtract)

                    # gather cell index: base + y0*187 + x0 (exact in fp32)
                    gfv = qp.tile([128, NS], FP32, tag="gfv")
                    nc.vector.scalar_tensor_tensor(out=gfv[:], in0=y0f[:], scalar=float(STRIDE), in1=tbase_sb[:], op0=AX.mult, op1=AX.add)
                    nc.vector.tensor_tensor(out=gfv[:], in0=gfv[:], in1=x0f[:], op=AX.add)
                    idxs = qp.tile([128, 2 * NS], INT32, tag="idxs")
                    nc.scalar.activation(out=idxs[:, 0::2], in_=gfv[:], func=AF.Copy)
                    nc.vector.tensor_scalar(out=idxs[:, 1::2], in0=gfv[:], scalar1=float(STRIDE), scalar2=None, op0=AX.add)

                    # fused bilinear*attention weights
                    tt = qp.tile([128, NS], FP32, tag="tt")
                    a0 = qp.tile([128, NS], FP32, tag="a0")
                    u0 = qp.tile([128, NS], FP32, tag="u0")
                    u1 = qp.tile([128, NS], FP32, tag="u1")
                    w00 = qp.tile([128, NS], BF16, tag="w00")
                    w01 = qp.tile([128, NS], BF16, tag="w01")
                    w10 = qp.tile([128, NS], BF16, tag="w10")
                    w11 = qp.tile([128, NS], BF16, tag="w11")
                    nc.vector.tensor_tensor(out=tt[:], in0=aw[:], in1=fy[:], op=AX.mult)
                    nc.vector.tensor_tensor(out=a0[:], in0=aw[:], in1=tt[:], op=AX.subtract)
                    nc.vector.tensor_tensor(out=u0[:], in0=a0[:], in1=fx[:], op=AX.mult)
                    nc.vector.tensor_tensor(out=u1[:], in0=tt[:], in1=fx[:], op=AX.mult)
                    nc.vector.tensor_tensor(out=w00[:], in0=a0[:], in1=u0[:], op=AX.subtract)
                    nc.vector.tensor_tensor(out=w10[:], in0=tt[:], in1=u1[:], op=AX.subtract)
                    nc.scalar.activation(out=w01[:], in_=u0[:], func=AF.Copy)
                    nc.scalar.activation(out=w11[:], in_=u1[:], func=AF.Copy)

                    # gather: per index 16 elems = 2 cells x 8 dh.
                    # one indirect DMA is limited to ~8191 descriptors (16-bit
                    # ring sem), so chunk the 320 per-row indices.
                    G = qg.tile([128, 2 * NS * 16], BF16, tag="G")
                    if no_gather:
                        nc.vector.memset(G[:], 0)
                    CHUNK = gather_chunk  # 128*63 = 8064 descriptors per call (limit ~8191)
                    idx_src = cidx_sb if const_idx else idxs
                    gtbl = XTBL if ext_table else V4
                    for c0 in ([] if no_gather else range(0, 2 * NS, CHUNK)):
                        c1 = min(c0 + CHUNK, 2 * NS)
                        nc.gpsimd.indirect_dma_start(
                            out=G[:, c0 * 16:c1 * 16], out_offset=None,
                            in_=gtbl[:],
                            in_offset=bass.IndirectOffsetOnAxis(ap=idx_src[:, c0:c1], axis=0),
                        )

                    # blend: m[p, h, d, lp] = sum_{r,c} w_rc * G[p,(h,lp),r,c,d]
                    gv = G[:].rearrange("p (h lp r c d) -> p h lp r c d", h=H, lp=L * NP, r=2, c=2, d=DH)
                    m = qp.tile([128, H * DH * L * NP], FP32, tag="m")
                    m2 = qp.tile([128, H * DH * L * NP], FP32, tag="m2")
                    mv = m[:].rearrange("p (h d lp) -> p h lp d", h=H, d=DH, lp=L * NP)
                    m2v = m2[:].rearrange("p (h d lp) -> p h lp d", h=H, d=DH, lp=L * NP)
                    def wb(w):
                        return w[:].rearrange("p (h lp one) -> p h lp one", h=H, one=1).to_broadcast((128, H, L * NP, DH))
                    nc.vector.tensor_tensor(out=mv, in0=gv[:, :, :, 0, 0, :], in1=wb(w00), op=AX.mult)
                    nc.vector.tensor_tensor(out=m2v, in0=gv[:, :, :, 0, 1, :], in1=wb(w01), op=AX.mult)
                    nc.vector.tensor_tensor(out=m[:], in0=m[:], in1=m2[:], op=AX.add)
                    nc.vector.tensor_tensor(out=m2v, in0=gv[:, :, :, 1, 0, :], in1=wb(w10), op=AX.mult)
                    nc.vector.tensor_tensor(out=m[:], in0=m[:], in1=m2[:], op=AX.add)
                    nc.vector.tensor_tensor(out=m2v, in0=gv[:, :, :, 1, 1, :], in1=wb(w11), op=AX.mult)
                    nc.vector.tensor_tensor(out=m[:], in0=m[:], in1=m2[:], op=AX.add)
                    attn = qp.tile([128, D], FP32, tag="attn")
                    nc.vector.tensor_reduce(
                        out=attn[:], in_=m[:].rearrange("p (hd lp) -> p hd lp", lp=L * NP),
                        axis=mybir.AxisListType.X, op=AX.add,
                    )

                    # output projection + residual + LN1
                    ps_t = qps.tile([64, 128], FP32, tag="ps_t")
                    nc.tensor.transpose(out=ps_t[:], in_=attn[:], identity=ident[:])
                    attnT = qp.tile([64, 128], FP32, tag="attnT")
                    nc.scalar.activation(out=attnT[:], in_=ps_t[:], func=AF.Copy)
                    ps_ao = qps.tile([128, D], FP32, tag="ps_ao")
                    nc.tensor.matmul(out=ps_ao[:], lhsT=attnT[:], rhs=Wout_sb[:], start=True, stop=True)
                    qft = qp.tile([128, D], FP32, tag="qft")
                    nc.sync.dma_start(qft[:], qf[qs, :])
                    xpre = qp.tile([128, D], FP32, tag="xpre")
                    nc.vector.tensor_tensor(out=xpre[:], in0=ps_ao[:], in1=bout_sb[:], op=AX.add)
                    nc.vector.tensor_tensor(out=xpre[:], in0=xpre[:], in1=qft[:], op=AX.add)

                    def layernorm(xin, gg, bb, xout_tag):
                        s1 = qp.tile([128, 1], FP32, tag=xout_tag + "_s1")
                        nc.vector.tensor_reduce(out=s1[:], in_=xin[:], axis=mybir.AxisListType.X, op=AX.add)
                        mn = qp.tile([128, 1], FP32, tag=xout_tag + "_mn")
                        nc.vector.tensor_scalar_mul(out=mn[:], in0=s1[:], scalar1=1.0 / 64.0)
                        xc = qp.tile([128, D], FP32, tag=xout_tag + "_xc")
                        nc.vector.tensor_tensor(out=xc[:], in0=xin[:], in1=mn[:].to_broadcast((128, D)), op=AX.subtract)
                        sq = qp.tile([128, D], FP32, tag=xout_tag + "_sq")
                        nc.scalar.activation(out=sq[:], in_=xc[:], func=AF.Square)
                        s2 = qp.tile([128, 1], FP32, tag=xout_tag + "_s2")
                        nc.vector.tensor_reduce(out=s2[:], in_=sq[:], axis=mybir.AxisListType.X, op=AX.add)
                        s2m = qp.tile([128, 1], FP32, tag=xout_tag + "_s2m")
                        nc.vector.tensor_scalar_mul(out=s2m[:], in0=s2[:], scalar1=1.0 / 64.0)
                        std = qp.tile([128, 1], FP32, tag=xout_tag + "_std")
                        nc.scalar.activation(out=std[:], in_=s2m[:], func=AF.Sqrt, bias=eps_sb[:])
                        rstd = qp.tile([128, 1], FP32, tag=xout_tag + "_rstd")
                        nc.vector.reciprocal(out=rstd[:], in_=std[:])
                        xo = qp.tile([128, D], FP32, tag=xout_tag)
                        nc.vector.tensor_tensor(out=xo[:], in0=xc[:], in1=rstd[:].to_broadcast((128, D)), op=AX.mult)
                        nc.vector.tensor_tensor(out=xo[:], in0=xo[:], in1=gg[:], op=AX.mult)
                        nc.vector.tensor_tensor(out=xo[:], in0=xo[:], in1=bb[:], op=AX.add)
                        return xo

                    x1 = layernorm(xpre, g1_sb, b1_sb, "x1")

                    # FFN
                    ps_t2 = qps.tile([64, 128], FP32, tag="ps_t2")
                    nc.tensor.transpose(out=ps_t2[:], in_=x1[:], identity=ident[:])
                    x1T = qp.tile([64, 128], FP32, tag="x1T")
                    nc.scalar.activation(out=x1T[:], in_=ps_t2[:], func=AF.Copy)
                    h1 = qp.tile([128, 1024], FP32, tag="h1")
                    for k in range(8):
                        ps_h1 = qps.tile([128, 128], FP32, tag="ps_h1")
                        nc.tensor.matmul(out=ps_h1[:], lhsT=W1_sb[:, k * 128:(k + 1) * 128], rhs=x1T[:], start=True, stop=True)
                        nc.scalar.activation(out=h1[:, k * 128:(k + 1) * 128], in_=ps_h1[:], func=AF.Relu, bias=bff1_sb[:, k:k + 1])
                    ps_h2 = qps.tile([128, D], FP32, tag="ps_h2")
                    for k in range(8):
                        nc.tensor.matmul(out=ps_h2[:], lhsT=h1[:, k * 128:(k + 1) * 128], rhs=W2_sb[:, k * D:(k + 1) * D], start=(k == 0), stop=(k == 7))
                    x2p = qp.tile([128, D], FP32, tag="x2p")
                    nc.vector.tensor_tensor(out=x2p[:], in0=ps_h2[:], in1=bff2_sb[:], op=AX.add)
                    nc.vector.tensor_tensor(out=x2p[:], in0=x2p[:], in1=x1[:], op=AX.add)
                    x2 = layernorm(x2p, g2_sb, b2_sb, "x2")
                    nc.sync.dma_start(OUT[qs, :], x2[:])

    nc.finalize()
    return nc


def _prep_core_inputs(q_feat_b, q_pos_b, ref_b, voxel_b, w):
    # q_* : [5000, 64] shards of one batch; voxel_b: [LIN, 64] of that batch
    qf = np.zeros((NQP, D), np.float32)
    qf[:q_feat_b.shape[0]] = q_feat_b
    qp = np.zeros((NQP, D), np.float32)
    qp[:q_pos_b.shape[0]] = q_pos_b
    rf = np.zeros((NQP, 2), np.float32)
    rf[:ref_b.shape[0]] = ref_b
    # padded-cell voxel layout with mask feature
    va = np.zeros((PADC, D + 1), np.float32)
    for l in range(L):
        hl, wl = SS[l]
        r0 = LBROW[l]
        blk = voxel_b[LSI[l]:LSI[l] + hl * wl].reshape(hl, wl, D)
        rows = (np.arange(hl) + r0 + 1)[:, None] * STRIDE + (np.arange(wl) + 1)[None, :]
        va[rows.ravel(), :D] = blk.reshape(-1, D)
        va[rows.ravel(), D] = 1.0
    m = {
        "vT": np.ascontiguousarray(va.T),
        "qfT": np.ascontiguousarray(qf.T),
        "qpT": np.ascontiguousarray(qp.T),
        "qf": qf,
        "ref": rf,
    }
    m.update(w)
    return m


def _weights_map(inputs):
    t_wl, t_hl, t_cxhi, t_cyhi, t_base = _build_tables()
    rep = lambda v: np.ascontiguousarray(np.broadcast_to(np.asarray(v, np.float32)[None, :], (128, len(v))))
    w = {
        "Wv": np.concatenate([np.asarray(inputs["Wv"], np.float32), np.asarray(inputs["bv"], np.float32)[None, :]], 0),
        "Wo": np.asarray(inputs["Wo"], np.float32),
        "Wa": np.asarray(inputs["Wa"], np.float32),
        "Wout": np.asarray(inputs["Wout"], np.float32),
        "W1": np.asarray(inputs["W1"], np.float32),
        "W2": np.ascontiguousarray(np.asarray(inputs["W2"], np.float32).reshape(8, 128, 64).transpose(1, 0, 2).reshape(128, 512)),
        "bo_r": rep(np.asarray(inputs["bo"], np.float32)),
        "ba_r": rep(np.asarray(inputs["ba"], np.float32)),
        "bout_r": rep(np.asarray(inputs["bout"], np.float32)),
        "g1_r": rep(np.asarray(inputs["g1"], np.float32)),
        "b1_r": rep(np.asarray(inputs["b1"], np.float32)),
        "g2_r": rep(np.asarray(inputs["g2"], np.float32)),
        "b2_r": rep(np.asarray(inputs["b2"], np.float32)),
        "bff1_c": np.ascontiguousarray(np.asarray(inputs["bff1"], np.float32).reshape(8, 128).T),
        "bff2_r": rep(np.asarray(inputs["bff2"], np.float32)),
        "t_wl": rep(t_wl), "t_hl": rep(t_hl), "t_cxhi": rep(t_cxhi),
        "t_cyhi": rep(t_cyhi), "t_base": rep(t_base),
    }
    return w


_NC_CACHE = {}


def kernel(**inputs) -> np.ndarray:
    if "nc" not in _NC_CACHE:
        _NC_CACHE["nc"] = build_program()
    nc = _NC_CACHE["nc"]
    w = _weights_map(inputs)
    q_feat = np.asarray(inputs["q_feat"], np.float32)
    q_pos = np.asarray(inputs["q_pos"], np.float32)
    ref = np.asarray(inputs["reference_points"], np.float32)
    vox = np.asarray(inputs["dense_voxel_flatten"], np.float32)
    QS = NQ // 4
    in_maps = []
    for c in range(8):
        b = c // 4
        s = slice((c % 4) * QS, (c % 4 + 1) * QS)
        in_maps.append(_prep_core_inputs(q_feat[b, s], q_pos[b, s], ref[b, s], vox[b], w))
    res = bass_utils.run_bass_kernel_spmd(nc, in_maps, core_ids=list(range(8)))
    out = np.zeros((B, NQ, D), np.float32)
    for c in range(8):
        b = c // 4
        s = slice((c % 4) * QS, (c % 4 + 1) * QS)
        out[b, s] = res.results[c]["out"][:QS]
    return out



# revision 12
# speedup vs baseline: 2555.3742x; 45.2205x over previous
import sys
sys.path.insert(0, '/opt/trn_rl_repo')
import numpy as np
import ml_dtypes

from concourse import bass, mybir, bacc
from concourse.tile import TileContext
from concourse.masks import make_identity
from concourse import bass_utils

# ---- problem constants (hardcoded) ----
D = 64
H = 8
L = 5
NP = 4            # points
DH = 8
NQ = 20000
B = 2
LIN = 45109
SS = [(184, 184), (92, 92), (46, 46), (23, 23), (12, 12)]   # (Hl, Wl)
LSI = [0, 33856, 42320, 44436, 44965]
STRIDE = 187                       # padded row stride in cells (>= Wl+3)
ROWS = [h + 3 for (h, w) in SS]    # rows incl. 1-top border + 2 bottom pad
CL = [r * STRIDE for r in ROWS]    # raw cells per level
CLP = [((c + 1023) // 1024) * 1024 for c in CL]   # level cells padded to groups
LBASE = [0]
for c in CLP[:-1]:
    LBASE.append(LBASE[-1] + c)
NCELL = sum(CLP)                   # 71680
NGRP = NCELL // 1024               # 70
NQP = 5120                         # queries per core (padded)
NT = NQP // 128                    # 40 query tiles
NS = H * L * NP                    # 160 sample slots per query
NGQ = 4                            # head-groups per tile (2 heads each)
SG = NS // NGQ                     # 40 slots per group
FP32 = mybir.dt.float32
BF16 = mybir.dt.bfloat16
INT32 = mybir.dt.int32
AX = mybir.AluOpType
AF = mybir.ActivationFunctionType


def _build_tables():
    # per-slot (j = h*20 + l*4 + p) constant rows
    t_wl = np.zeros(NS, np.float32)
    t_hl = np.zeros(NS, np.float32)
    t_cxhi = np.zeros(NS, np.float32)
    t_cyhi = np.zeros(NS, np.float32)
    t_base = np.zeros(NS, np.float32)
    for h in range(H):
        for l in range(L):
            hl, wl = SS[l]
            for p in range(NP):
                j = h * (L * NP) + l * NP + p
                t_wl[j] = wl
                t_hl[j] = hl
                t_cxhi[j] = wl + 1
                t_cyhi[j] = hl + 1
                t_base[j] = LBASE[l]
    return t_wl, t_hl, t_cxhi, t_cyhi, t_base


def build_program(nt=NT, gcols=40, no_gather=False, no_vpipe=False,
                  no_blend=False, no_tail=False, no_pos=False):
    nc = bacc.Bacc()
    dt = nc.dram_tensor
    vT = dt("vT", (D + 1, NCELL), BF16, kind="ExternalInput")
    qsT = dt("qsT", (D, NQP), BF16, kind="ExternalInput")
    qf = dt("qf", (NQP, D), FP32, kind="ExternalInput")
    ref = dt("ref", (NQP, 2), FP32, kind="ExternalInput")
    Wv = dt("Wv", (D + 1, D), BF16, kind="ExternalInput")
    Wo = dt("Wo", (D, H * L * NP * 2), BF16, kind="ExternalInput")
    Wa = dt("Wa", (D, NS), BF16, kind="ExternalInput")
    Wout = dt("Wout", (D, D), FP32, kind="ExternalInput")
    W1 = dt("W1", (D, 1024), FP32, kind="ExternalInput")
    W2 = dt("W2", (128, 8 * D), FP32, kind="ExternalInput")
    bo_r = dt("bo_r", (128, 320), FP32, kind="ExternalInput")
    ba_r = dt("ba_r", (128, NS), FP32, kind="ExternalInput")
    bout_r = dt("bout_r", (128, D), FP32, kind="ExternalInput")
    g1_r = dt("g1_r", (128, D), FP32, kind="ExternalInput")
    b1_r = dt("b1_r", (128, D), FP32, kind="ExternalInput")
    g2_r = dt("g2_r", (128, D), FP32, kind="ExternalInput")
    b2_r = dt("b2_r", (128, D), FP32, kind="ExternalInput")
    bff1_c = dt("bff1_c", (128, 8), FP32, kind="ExternalInput")
    bff2_r = dt("bff2_r", (128, D), FP32, kind="ExternalInput")
    t_wl = dt("t_wl", (128, NS), FP32, kind="ExternalInput")
    t_hl = dt("t_hl", (128, NS), FP32, kind="ExternalInput")
    t_cxhi = dt("t_cxhi", (128, NS), FP32, kind="ExternalInput")
    t_cyhi = dt("t_cyhi", (128, NS), FP32, kind="ExternalInput")
    t_base = dt("t_base", (128, NS), FP32, kind="ExternalInput")
    OUT = dt("out", (NQP, D), FP32, kind="ExternalOutput")
    TBL = dt("tbl", (NCELL * D,), BF16, kind="Internal")
    TBLv = TBL[:].rearrange("(c d) -> c d", d=D)

    with TileContext(nc) as tc:
        with tc.tile_pool(name="const", bufs=1) as cp:
            def ld(src, shape, dtype=FP32):
                t = cp.tile(shape, dtype, tag=src.name + "_sb")
                nc.sync.dma_start(t[:], src[:])
                return t
            Wv_sb = ld(Wv, [D + 1, D], BF16)
            Wo_sb = ld(Wo, [D, 320], BF16)
            Wa_sb = ld(Wa, [D, NS], BF16)
            Wout_sb = ld(Wout, [D, D])
            W1_sb = ld(W1, [D, 1024])
            W2_sb = ld(W2, [128, 8 * D])
            bo_sb = ld(bo_r, [128, 320])
            ba_sb = ld(ba_r, [128, NS])
            bout_sb = ld(bout_r, [128, D])
            g1_sb = ld(g1_r, [128, D])
            b1_sb = ld(b1_r, [128, D])
            g2_sb = ld(g2_r, [128, D])
            b2_sb = ld(b2_r, [128, D])
            bff1_sb = ld(bff1_c, [128, 8])
            bff2_sb = ld(bff2_r, [128, D])
            twl_sb = ld(t_wl, [128, NS])
            thl_sb = ld(t_hl, [128, NS])
            tcx_sb = ld(t_cxhi, [128, NS])
            tcy_sb = ld(t_cyhi, [128, NS])
            tbase_sb = ld(t_base, [128, NS])
            eps_sb = cp.tile([128, 1], FP32, tag="eps")
            nc.vector.memset(eps_sb[:], 1e-5)
            ident = cp.tile([128, 128], FP32, tag="ident")
            make_identity(nc, ident[:])
            # qT = (q_feat + q_pos)^T, host-precomputed in bf16
            qT = cp.tile([D, NQP], BF16, tag="qT")
            nc.sync.dma_start(qT[:], qsT[:])
            with tc.tile_pool(name="vload", bufs=3) as vl, \
                 tc.tile_pool(name="vstg", bufs=3) as vstg, \
                 tc.tile_pool(name="vps", bufs=4, space="PSUM") as vps:
                # ---------- value pipeline ----------
                # host permuted vT columns: col g*1024 + j*128 + p <-> cell g*1024 + p*8 + j
                for g2 in range(0 if no_vpipe else NGRP // 2):     # load 2 groups per DMA
                    vchunk = vl.tile([D + 1, 2048], BF16, tag="vchunk")
                    nc.sync.dma_start(vchunk[:], vT[:, g2 * 2048:(g2 + 1) * 2048])
                    for gg in range(2):
                        g = g2 * 2 + gg
                        ps = vps.tile([128, 512], FP32, tag="vps")
                        for j in range(8):
                            nc.tensor.matmul(
                                out=ps[:, j * 64:(j + 1) * 64],
                                lhsT=vchunk[:, gg * 1024 + j * 128: gg * 1024 + (j + 1) * 128],
                                rhs=Wv_sb[:],
                                start=True, stop=True,
                            )
                        stg = vstg.tile([128, 512], BF16, tag="vstg")
                        nc.scalar.activation(out=stg[:], in_=ps[:], func=AF.Copy)
                        dst = TBL[g * 65536:(g + 1) * 65536].rearrange("(p f) -> p f", p=128)
                        nc.sync.dma_start(dst, stg[:])

            # ---------- query loop ----------
            with tc.tile_pool(name="qw", bufs=2) as qp, \
                 tc.tile_pool(name="qg", bufs=3) as qg, \
                 tc.tile_pool(name="qps", bufs=1, space="PSUM") as qps:
                for t in range(nt):
                    qs = slice(t * 128, (t + 1) * 128)
                    # attention weights (softmax over 20 per head)
                    ps_aw = qps.tile([128, NS], FP32, tag="ps_aw")
                    nc.tensor.matmul(out=ps_aw[:], lhsT=qT[:, qs], rhs=Wa_sb[:], start=True, stop=True)
                    logit = qp.tile([128, NS], FP32, tag="logit")
                    nc.vector.tensor_tensor(out=logit[:], in0=ps_aw[:], in1=ba_sb[:], op=AX.add)
                    mx = qp.tile([128, H], FP32, tag="mx")
                    lv = logit[:].rearrange("p (h k) -> p h k", h=H)
                    nc.vector.tensor_reduce(out=mx[:], in_=lv, axis=mybir.AxisListType.X, op=AX.max)
                    mxb = mx[:].rearrange("p (h one) -> p h one", one=1).to_broadcast((128, H, L * NP))
                    ls = qp.tile([128, NS], FP32, tag="ls")
                    nc.vector.tensor_tensor(out=ls[:].rearrange("p (h k) -> p h k", h=H), in0=lv, in1=mxb, op=AX.subtract)
                    ee = qp.tile([128, NS], FP32, tag="ee")
                    nc.scalar.activation(out=ee[:], in_=ls[:], func=AF.Exp)
                    sm = qp.tile([128, H], FP32, tag="sm")
                    nc.vector.tensor_reduce(out=sm[:], in_=ee[:].rearrange("p (h k) -> p h k", h=H), axis=mybir.AxisListType.X, op=AX.add)
                    rc = qp.tile([128, H], FP32, tag="rc")
                    nc.vector.reciprocal(out=rc[:], in_=sm[:])
                    aw = qp.tile([128, NS], FP32, tag="aw")
                    rcb = rc[:].rearrange("p (h one) -> p h one", one=1).to_broadcast((128, H, L * NP))
                    nc.vector.tensor_tensor(out=aw[:].rearrange("p (h k) -> p h k", h=H), in0=ee[:].rearrange("p (h k) -> p h k", h=H), in1=rcb, op=AX.mult)

                    # sampling offsets
                    ps_off = qps.tile([128, 320], FP32, tag="ps_off")
                    nc.tensor.matmul(out=ps_off[:], lhsT=qT[:, qs], rhs=Wo_sb[:], start=True, stop=True)
                    off = qp.tile([128, 320], FP32, tag="off")
                    nc.vector.tensor_tensor(out=off[:], in0=ps_off[:], in1=bo_sb[:], op=AX.add)

                    reft = qp.tile([128, 2], FP32, tag="reft")
                    nc.sync.dma_start(reft[:], ref[qs, :])
                    refx = reft[:, 0:1].to_broadcast((128, NS))
                    refy = reft[:, 1:2].to_broadcast((128, NS))

                    # positions: p = ref*W + off + 0.5, clamp [0, W+1]
                    tmp = qp.tile([128, NS], FP32, tag="tmp")
                    pxc = qp.tile([128, NS], FP32, tag="pxc")
                    pyc = qp.tile([128, NS], FP32, tag="pyc")
                    nc.vector.tensor_tensor(out=tmp[:], in0=refx, in1=twl_sb[:], op=AX.mult)
                    nc.vector.scalar_tensor_tensor(out=tmp[:], in0=off[:, 0::2], scalar=0.5, in1=tmp[:], op0=AX.add, op1=AX.add)
                    nc.vector.scalar_tensor_tensor(out=pxc[:], in0=tmp[:], scalar=0.0, in1=tcx_sb[:], op0=AX.max, op1=AX.min)
                    nc.vector.tensor_tensor(out=tmp[:], in0=refy, in1=thl_sb[:], op=AX.mult)
                    nc.vector.scalar_tensor_tensor(out=tmp[:], in0=off[:, 1::2], scalar=0.5, in1=tmp[:], op0=AX.add, op1=AX.add)
                    nc.vector.scalar_tensor_tensor(out=pyc[:], in0=tmp[:], scalar=0.0, in1=tcy_sb[:], op0=AX.max, op1=AX.min)

                    x0i = qp.tile([128, NS], INT32, tag="x0i")
                    x0f = qp.tile([128, NS], FP32, tag="x0f")
                    y0i = qp.tile([128, NS], INT32, tag="y0i")
                    y0f = qp.tile([128, NS], FP32, tag="y0f")
                    nc.scalar.activation(out=x0i[:], in_=pxc[:], func=AF.Copy)
                    nc.scalar.activation(out=x0f[:], in_=x0i[:], func=AF.Copy)
                    nc.scalar.activation(out=y0i[:], in_=pyc[:], func=AF.Copy)
                    nc.scalar.activation(out=y0f[:], in_=y0i[:], func=AF.Copy)
                    fx = qp.tile([128, NS], FP32, tag="fx")
                    fy = qp.tile([128, NS], FP32, tag="fy")
                    nc.vector.tensor_tensor(out=fx[:], in0=pxc[:], in1=x0f[:], op=AX.subtract)
                    nc.vector.tensor_tensor(out=fy[:], in0=pyc[:], in1=y0f[:], op=AX.subtract)

                    # gather cell index: base + y0*187 + x0 (exact in fp32)
                    gfv = qp.tile([128, NS], FP32, tag="gfv")
                    nc.vector.scalar_tensor_tensor(out=gfv[:], in0=y0f[:], scalar=float(STRIDE), in1=tbase_sb[:], op0=AX.mult, op1=AX.add)
                    nc.vector.tensor_tensor(out=gfv[:], in0=gfv[:], in1=x0f[:], op=AX.add)
                    idxs = qp.tile([128, 2 * NS], INT32, tag="idxs")
                    nc.scalar.activation(out=idxs[:, 0::2], in_=gfv[:], func=AF.Copy)
                    nc.vector.tensor_scalar(out=idxs[:, 1::2], in0=gfv[:], scalar1=float(STRIDE), scalar2=None, op0=AX.add)

                    # bilinear x attention weights (fp32)
                    tt = qp.tile([128, NS], FP32, tag="tt")
                    a0 = qp.tile([128, NS], FP32, tag="a0")
                    w00 = qp.tile([128, NS], FP32, tag="w00")
                    w01 = qp.tile([128, NS], FP32, tag="w01")
                    w10 = qp.tile([128, NS], FP32, tag="w10")
                    w11 = qp.tile([128, NS], FP32, tag="w11")
                    nc.vector.tensor_tensor(out=tt[:], in0=aw[:], in1=fy[:], op=AX.mult)
                    nc.vector.tensor_tensor(out=a0[:], in0=aw[:], in1=tt[:], op=AX.subtract)
                    nc.vector.tensor_tensor(out=w01[:], in0=a0[:], in1=fx[:], op=AX.mult)
                    nc.vector.tensor_tensor(out=w11[:], in0=tt[:], in1=fx[:], op=AX.mult)
                    nc.vector.tensor_tensor(out=w00[:], in0=a0[:], in1=w01[:], op=AX.subtract)
                    nc.vector.tensor_tensor(out=w10[:], in0=tt[:], in1=w11[:], op=AX.subtract)

                    # per head-group: gather then blend
                    m = qp.tile([128, H * (L * NP) * DH], FP32, tag="m")
                    tmpb = qp.tile([128, 2 * (L * NP) * DH], FP32, tag="tmpb")
                    for g in range(NGQ):
                        G = qg.tile([128, 2 * SG * 128], BF16, tag="G")
                        if no_gather:
                            nc.vector.memset(G[:], 0)
                        for c0 in ([] if no_gather else range(0, 2 * SG, gcols)):
                            c1 = min(c0 + gcols, 2 * SG)
                            nc.gpsimd.indirect_dma_start(
                                out=G[:, c0 * 128:c1 * 128], out_offset=None,
                                in_=TBLv,
                                in_offset=bass.IndirectOffsetOnAxis(ap=idxs[:, 2 * SG * g + c0: 2 * SG * g + c1], axis=0),
                            )
                        if no_blend:
                            if g == 0:
                                nc.vector.memset(m[:], 0)
                            nc.vector.tensor_tensor(
                                out=m[:, g * 64:g * 64 + 64],
                                in0=m[:, g * 64:g * 64 + 64], in1=G[:, 0:64], op=AX.add)
                            continue
                        # blend: m[p, h2, lp, d] += w_rc * G[p, ((h2,lp)*2+r)*128 + c*64 + h2*8 + d]
                        base = G[:]
                        ms = m[:, g * 2 * (L * NP) * DH:(g + 1) * 2 * (L * NP) * DH]
                        msv = ms.rearrange("p (h lp d) -> p h lp d", h=2, d=DH)
                        tbv = tmpb[:].rearrange("p (h lp d) -> p h lp d", h=2, d=DH)

                        def gview(r, cx, h2base):
                            off_e = r * 128 + cx * 64 + h2base * 8
                            ap = [list(base.ap[0]), [2 * (L * NP) * 128 + 8, 2],
                                  [2 * 128, L * NP], [1, DH]]
                            return bass.AP(base.tensor, base.offset + off_e, ap)

                        def wview(w, r, cx):
                            ws = w[:, g * SG:(g + 1) * SG]
                            return ws.rearrange("p (h lp) -> p h lp", h=2).unsqueeze(3).to_broadcast((128, 2, L * NP, DH))

                        h2b = 2 * g
                        nc.vector.tensor_tensor(out=msv, in0=gview(0, 0, h2b), in1=wview(w00, 0, 0), op=AX.mult)
                        nc.vector.tensor_tensor(out=tbv, in0=gview(0, 1, h2b), in1=wview(w01, 0, 1), op=AX.mult)
                        nc.vector.tensor_tensor(out=ms, in0=ms, in1=tmpb[:], op=AX.add)
                        nc.vector.tensor_tensor(out=tbv, in0=gview(1, 0, h2b), in1=wview(w10, 1, 0), op=AX.mult)
                        nc.vector.tensor_tensor(out=ms, in0=ms, in1=tmpb[:], op=AX.add)
                        nc.vector.tensor_tensor(out=tbv, in0=gview(1, 1, h2b), in1=wview(w11, 1, 1), op=AX.mult)
                        nc.vector.tensor_tensor(out=ms, in0=ms, in1=tmpb[:], op=AX.add)

                    attn = qp.tile([128, D], FP32, tag="attn")
                    nc.vector.tensor_reduce(
                        out=attn[:].rearrange("p (h d) -> p h d", h=H),
                        in_=m[:].rearrange("p (h lp d) -> p h d lp", h=H, d=DH),
                        axis=mybir.AxisListType.X, op=AX.add,
                    )

                    if no_tail:
                        nc.sync.dma_start(OUT[qs, :], attn[:])
                        continue

                    # output projection + residual + LN1
                    ps_t = qps.tile([64, 128], FP32, tag="ps_t")
                    nc.tensor.transpose(out=ps_t[:], in_=attn[:], identity=ident[:])
                    attnT = qp.tile([64, 128], FP32, tag="attnT")
                    nc.scalar.activation(out=attnT[:], in_=ps_t[:], func=AF.Copy)
                    ps_ao = qps.tile([128, D], FP32, tag="ps_ao")
                    nc.tensor.matmul(out=ps_ao[:], lhsT=attnT[:], rhs=Wout_sb[:], start=True, stop=True)
                    qft = qp.tile([128, D], FP32, tag="qft")
                    nc.sync.dma_start(qft[:], qf[qs, :])
                    xpre = qp.tile([128, D], FP32, tag="xpre")
                    nc.vector.tensor_tensor(out=xpre[:], in0=ps_ao[:], in1=bout_sb[:], op=AX.add)
                    nc.vector.tensor_tensor(out=xpre[:], in0=xpre[:], in1=qft[:], op=AX.add)

                    def layernorm(xin, gg, bb, xout_tag):
                        s1 = qp.tile([128, 1], FP32, tag=xout_tag + "_s1")
                        nc.vector.tensor_reduce(out=s1[:], in_=xin[:], axis=mybir.AxisListType.X, op=AX.add)
                        mn = qp.tile([128, 1], FP32, tag=xout_tag + "_mn")
                        nc.vector.tensor_scalar_mul(out=mn[:], in0=s1[:], scalar1=1.0 / 64.0)
                        xc = qp.tile([128, D], FP32, tag=xout_tag + "_xc")
                        nc.vector.tensor_tensor(out=xc[:], in0=xin[:], in1=mn[:].to_broadcast((128, D)), op=AX.subtract)
                        sq = qp.tile([128, D], FP32, tag=xout_tag + "_sq")
                        nc.scalar.activation(out=sq[:], in_=xc[:], func=AF.Square)
                        s2 = qp.tile([128, 1], FP32, tag=xout_tag + "_s2")
                        nc.vector.tensor_reduce(out=s2[:], in_=sq[:], axis=mybir.AxisListType.X, op=AX.add)
                        s2m = qp.tile([128, 1], FP32, tag=xout_tag + "_s2m")
                        nc.vector.tensor_scalar_mul(out=s2m[:], in0=s2[:], scalar1=1.0 / 64.0)
                        std = qp.tile([128, 1], FP32, tag=xout_tag + "_std")
                        nc.scalar.activation(out=std[:], in_=s2m[:], func=AF.Sqrt, bias=eps_sb[:])
                        rstd = qp.tile([128, 1], FP32, tag=xout_tag + "_rstd")
                        nc.vector.reciprocal(out=rstd[:], in_=std[:])
                        xo = qp.tile([128, D], FP32, tag=xout_tag)
                        nc.vector.tensor_tensor(out=xo[:], in0=xc[:], in1=rstd[:].to_broadcast((128, D)), op=AX.mult)
                        nc.vector.tensor_tensor(out=xo[:], in0=xo[:], in1=gg[:], op=AX.mult)
                        nc.vector.tensor_tensor(out=xo[:], in0=xo[:], in1=bb[:], op=AX.add)
                        return xo

                    x1 = layernorm(xpre, g1_sb, b1_sb, "x1")

                    # FFN
                    ps_t2 = qps.tile([64, 128], FP32, tag="ps_t2")
                    nc.tensor.transpose(out=ps_t2[:], in_=x1[:], identity=ident[:])
                    x1T = qp.tile([64, 128], FP32, tag="x1T")
                    nc.scalar.activation(out=x1T[:], in_=ps_t2[:], func=AF.Copy)
                    h1 = qp.tile([128, 1024], FP32, tag="h1")
                    for k in range(8):
                        ps_h1 = qps.tile([128, 128], FP32, tag="ps_h1")
                        nc.tensor.matmul(out=ps_h1[:], lhsT=W1_sb[:, k * 128:(k + 1) * 128], rhs=x1T[:], start=True, stop=True)
                        nc.scalar.activation(out=h1[:, k * 128:(k + 1) * 128], in_=ps_h1[:], func=AF.Relu, bias=bff1_sb[:, k:k + 1])
                    ps_h2 = qps.tile([128, D], FP32, tag="ps_h2")
                    for k in range(8):
                        nc.tensor.matmul(out=ps_h2[:], lhsT=h1[:, k * 128:(k + 1) * 128], rhs=W2_sb[:, k * D:(k + 1) * D], start=(k == 0), stop=(k == 7))
                    x2p = qp.tile([128, D], FP32, tag="x2p")
                    nc.vector.tensor_tensor(out=x2p[:], in0=ps_h2[:], in1=bff2_sb[:], op=AX.add)
                    nc.vector.tensor_tensor(out=x2p[:], in0=x2p[:], in1=x1[:], op=AX.add)
                    x2 = layernorm(x2p, g2_sb, b2_sb, "x2")
                    nc.sync.dma_start(OUT[qs, :], x2[:])

    nc.finalize()
    return nc


def _prep_vT(voxel_b):
    # padded per-level grids, all-heads-per-cell, group-permuted columns,
    # with mask lane D for bias gating
    va = np.zeros((NCELL, D + 1), np.float32)
    for l in range(L):
        hl, wl = SS[l]
        blk = voxel_b[LSI[l]:LSI[l] + hl * wl].reshape(hl, wl, D)
        rows = (np.arange(hl) + 1)[:, None] * STRIDE + (np.arange(wl) + 1)[None, :] + LBASE[l]
        va[rows.ravel(), :D] = blk.reshape(-1, D)
        va[rows.ravel(), D] = 1.0
    # permute: column g*1024 + j*128 + p  <-  cell g*1024 + p*8 + j
    vp = va.reshape(NGRP, 128, 8, D + 1).transpose(0, 2, 1, 3).reshape(NCELL, D + 1)
    return np.ascontiguousarray(vp.T).astype(ml_dtypes.bfloat16)


def _prep_core_inputs(q_feat_b, q_pos_b, ref_b, voxel_b, w):
    qfa = np.zeros((NQP, D), np.float32)
    qfa[:q_feat_b.shape[0]] = q_feat_b
    qpa = np.zeros((NQP, D), np.float32)
    qpa[:q_pos_b.shape[0]] = q_pos_b
    rf = np.zeros((NQP, 2), np.float32)
    rf[:ref_b.shape[0]] = ref_b
    m = {
        "vT": _prep_vT(voxel_b),
        "qsT": np.ascontiguousarray((qfa + qpa).T).astype(ml_dtypes.bfloat16),
        "qf": qfa,
        "ref": rf,
    }
    m.update(w)
    return m


def _weights_map(inputs):
    t_wl, t_hl, t_cxhi, t_cyhi, t_base = _build_tables()
    rep = lambda v: np.ascontiguousarray(np.broadcast_to(np.asarray(v, np.float32)[None, :], (128, len(v))))
    w = {
        "Wv": np.concatenate([np.asarray(inputs["Wv"], np.float32), np.asarray(inputs["bv"], np.float32)[None, :]], 0).astype(ml_dtypes.bfloat16),
        "Wo": np.asarray(inputs["Wo"], np.float32).astype(ml_dtypes.bfloat16),
        "Wa": np.asarray(inputs["Wa"], np.float32).astype(ml_dtypes.bfloat16),
        "Wout": np.asarray(inputs["Wout"], np.float32),
        "W1": np.asarray(inputs["W1"], np.float32),
        "W2": np.ascontiguousarray(np.asarray(inputs["W2"], np.float32).reshape(8, 128, 64).transpose(1, 0, 2).reshape(128, 512)),
        "bo_r": rep(np.asarray(inputs["bo"], np.float32)),
        "ba_r": rep(np.asarray(inputs["ba"], np.float32)),
        "bout_r": rep(np.asarray(inputs["bout"], np.float32)),
        "g1_r": rep(np.asarray(inputs["g1"], np.float32)),
        "b1_r": rep(np.asarray(inputs["b1"], np.float32)),
        "g2_r": rep(np.asarray(inputs["g2"], np.float32)),
        "b2_r": rep(np.asarray(inputs["b2"], np.float32)),
        "bff1_c": np.ascontiguousarray(np.asarray(inputs["bff1"], np.float32).reshape(8, 128).T),
        "bff2_r": rep(np.asarray(inputs["bff2"], np.float32)),
        "t_wl": rep(t_wl), "t_hl": rep(t_hl), "t_cxhi": rep(t_cxhi),
        "t_cyhi": rep(t_cyhi), "t_base": rep(t_base),
    }
    return w


_NC_CACHE = {}


def kernel(**inputs) -> np.ndarray:
    if "nc" not in _NC_CACHE:
        _NC_CACHE["nc"] = build_program()
    nc = _NC_CACHE["nc"]
    w = _weights_map(inputs)
    q_feat = np.asarray(inputs["q_feat"], np.float32)
    q_pos = np.asarray(inputs["q_pos"], np.float32)
    ref = np.asarray(inputs["reference_points"], np.float32)
    vox = np.asarray(inputs["dense_voxel_flatten"], np.float32)
    QS = NQ // 4
    in_maps = []
    for c in range(8):
        b = c // 4
        s = slice((c % 4) * QS, (c % 4 + 1) * QS)
        in_maps.append(_prep_core_inputs(q_feat[b, s], q_pos[b, s], ref[b, s], vox[b], w))
    res = bass_utils.run_bass_kernel_spmd(nc, in_maps, core_ids=list(range(8)))
    out = np.zeros((B, NQ, D), np.float32)
    for c in range(8):
        b = c // 4
        s = slice((c % 4) * QS, (c % 4 + 1) * QS)
        out[b, s] = res.results[c]["out"][:QS]
    return out
